# revision 1
# baseline (speedup 1.0000x reference)
"""Trainium2 Bass kernel for a 3-layer dense GCN (nn_DenseGCN3Layer).

Node/dst sharding over 8 NeuronCores. Per layer: each core projects its
local shard to table_l = dinv*(x_{l-1} @ W_l) in bf16, AllGathers the full
table into local HBM, then fetches per-edge 256B source rows with 4-queue
SWDGE dma_gather; TensorE one-hot matmuls segment-sum edges into PSUM per
128-dst-node block; DVE/ACT finish ops apply dinv, bias, skips, ReLU.
Host work is integer edge indexing/sorting, dtype casts, layout packing.
"""
import numpy as np
import ml_dtypes

import concourse.bacc as bacc
import concourse.bass as bass
import concourse.mybir as mybir
from concourse._compat import cdiv
from concourse.bass_utils import run_bass_kernel_spmd
from concourse.library_config import mlp

bf16 = ml_dtypes.bfloat16
f32 = mybir.dt.float32
bfl = mybir.dt.bfloat16
i16 = mybir.dt.int16
AF = mybir.ActivationFunctionType
OP = mybir.AluOpType
AX = mybir.AxisListType

N_CORES = 8
P = 128
NOH = 40    # one-hot buffers
NGB = 4     # gather buffers
NIB = 5     # idx slice buffers
MAXG = 4608  # max slots per gather instruction
GCH = MAXG // P
Fs = [64, 32, 16]


def _prep(edge_index, n_nodes):
    src = np.asarray(edge_index[0], np.int64)
    dst = np.asarray(edge_index[1], np.int64)
    loops = np.arange(n_nodes, dtype=np.int64)
    src = np.concatenate([src, loops])
    dst = np.concatenate([dst, loops])
    deg = np.bincount(dst, minlength=n_nodes).astype(np.float64)

    shard = n_nodes // N_CORES
    nodep = cdiv(shard, P) * P
    if nodep == shard:
        nodep += P
    n_blocks = nodep // P
    quad_rows = 2 * nodep
    assert quad_rows < 32768
    nq = 4

    core_of = dst // shard
    per_core = []
    run_len = np.zeros((n_blocks, nq), np.int64)
    for c in range(N_CORES):
        m = core_of == c
        s = src[m]
        dloc = dst[m] - c * shard
        blk = dloc // P
        q = s // (2 * shard)
        idxq = ((s // shard) % 2) * nodep + (s % shard)
        order = np.lexsort((idxq, q, blk))
        blk, q, idxq, dloc = blk[order], q[order], idxq[order], dloc[order]
        runs = {}
        for b in range(n_blocks):
            mb = blk == b
            qb, ib, db = q[mb], idxq[mb], dloc[mb] % P
            for qq in range(nq):
                m2 = qb == qq
                runs[(b, qq)] = (ib[m2], db[m2])
                run_len[b, qq] = max(run_len[b, qq], int(m2.sum()))
        deg_c = np.full(nodep, 1e30)
        deg_c[:shard] = deg[c * shard:(c + 1) * shard]
        per_core.append((runs, deg_c))

    run_pad = ((run_len + P) // P) * P

    sbs, cur, cur_tot = [], [], 0
    for b in range(n_blocks):
        mx = int(run_pad[b].max())
        if cur and cur_tot + mx > MAXG:
            sbs.append(cur)
            cur, cur_tot = [], 0
        cur.append(b)
        cur_tot += mx
    if cur:
        sbs.append(cur)

    gathers, sched = [], []
    for sb in sbs:
        for qq in range(nq):
            tot = 0
            for b in sb:
                npad = int(run_pad[b, qq])
                tot += npad
                for ci in range(npad // P):
                    sched.append((b, qq == 0 and ci == 0,
                                  qq == nq - 1 and ci == npad // P - 1))
            gathers.append((qq, tot))

    cores = []
    for c in range(N_CORES):
        runs, deg_c = per_core[c]
        idx_s, dstl_s = [], []
        for sb in sbs:
            for qq in range(nq):
                for b in sb:
                    ib, db = runs[(b, qq)]
                    pad = int(run_pad[b, qq]) - len(ib)
                    idx_s.append(np.concatenate(
                        [ib, np.full(pad, nodep - 1, np.int64)]))
                    dstl_s.append(np.concatenate(
                        [db, np.full(pad, 254, np.int64)]))
        idx_s = np.concatenate(idx_s)
        dstl_s = np.concatenate(dstl_s)
        w = idx_s.reshape(-1, 16).T.astype(np.int16)
        cores.append(dict(
            idx=np.ascontiguousarray(np.tile(w, (8, 1))),
            dstl=np.ascontiguousarray(
                dstl_s.reshape(-1, P).T.astype(np.float32)),
            deg=np.ascontiguousarray(
                deg_c.reshape(n_blocks, P).T.astype(np.float32)),
        ))
    common = dict(gathers=gathers, sched=sched, n_slots=len(idx_s),
                  shard=shard, nodep=nodep, n_blocks=n_blocks, nq=nq,
                  quad_rows=quad_rows)
    return cores, common


def build_program(common, F_IN):
    nodep = common["nodep"]
    n_blocks, nq = common["n_blocks"], common["nq"]
    gathers, sched = common["gathers"], common["sched"]
    n_slots = common["n_slots"]
    n_chunks = n_slots // P
    quad_rows = common["quad_rows"]
    KT = F_IN // P
    assert F_IN % P == 0
    n_gath = len(gathers)
    nb = n_blocks
    ICOL = MAXG // 16

    gpfx = [0]
    for (_, tot) in gathers:
        gpfx.append(gpfx[-1] + tot // P)
    assert gpfx[-1] == n_chunks
    g_of_chunk = []
    for g, (_, tot) in enumerate(gathers):
        g_of_chunk += [g] * (tot // P)
    # maximal consecutive same-(gather, block) runs; PE waits s_oh per run
    run_start = []
    run_end_of = [0] * n_chunks
    i = 0
    while i < n_chunks:
        j = i
        while (j + 1 < n_chunks and g_of_chunk[j + 1] == g_of_chunk[i]
               and sched[j + 1][0] == sched[i][0]):
            j += 1
        for k in range(i, j + 1):
            run_end_of[k] = j
        run_start.append(i)
        i = j + 1
    run_start = set(run_start)
    assert max(run_end_of[i] - i for i in run_start) < NOH - 2

    # ---------------- pre-pass: event tables for sem targets -------------
    # pem: PE misc ops (stage1 block-mms count 1, transposes, projections)
    # pj:  ps_pj bank uses (global, ping-pong over 2 banks)
    # tpd: transpose copy-out count; relu: finish relu count
    # dfin: DVE finish-chain inc count; stg: staging-write count
    PEM_ST1 = {}
    PJ_ST1 = {}
    PEM_TP = {}
    PEM_P1 = {}
    PEM_P2 = {}
    PJ1 = {}
    PJ2 = {}
    DF_A = {}
    DF_B = {}
    pem = pj = dfin = 0
    for b in range(nb):
        pem += 1
        PEM_ST1[b] = pem
        PJ_ST1[b] = pj
        pj += 1
    fins = [[] for _ in range(3)]
    for l in range(3):
        fp = 0
        for ci, (b, st, fi) in enumerate(sched):
            if fi:
                fins[l].append((b, ci))
                gl = l * nb + fp
                dfin += 1
                DF_A[gl] = dfin
                if l < 2:
                    pem += 1
                    PEM_TP[gl] = pem
                    PJ1[gl] = pj
                    pj += 1
                    pem += 1
                    PEM_P1[gl] = pem
                    if l == 0:
                        PJ2[gl] = pj
                        pj += 1
                        pem += 1
                        PEM_P2[gl] = pem
                else:
                    dfin += 1
                    DF_B[gl] = dfin
                fp += 1

    def bank_of(l, b):
        return (l * nb + b) % 5

    def bank_use(l, b):
        return (l * nb + b) // 5

    nc = bacc.Bacc("TRN2", target_bir_lowering=False, debug=False,
                   num_devices=N_CORES, num_swdge_queues=4)

    xt_d = nc.dram_tensor("xt", [F_IN, nodep], bfl, kind="ExternalInput")
    idx_d = nc.dram_tensor("idx", [P, n_slots // 16], i16, kind="ExternalInput")
    dstl_d = nc.dram_tensor("dstl", [P, n_chunks], f32, kind="ExternalInput")
    deg_d = nc.dram_tensor("deg", [P, nb], f32, kind="ExternalInput")
    wall_d = nc.dram_tensor("wall", [F_IN, 112], bfl, kind="ExternalInput")
    w2_d = nc.dram_tensor("w2", [64, 32], bfl, kind="ExternalInput")
    w3_d = nc.dram_tensor("w3", [32, 16], bfl, kind="ExternalInput")
    ws13_d = nc.dram_tensor("ws13", [64, 16], bfl, kind="ExternalInput")
    bias_d = nc.dram_tensor("bias", [P, 176], f32, kind="ExternalInput")
    bout_d = nc.dram_tensor("bout", [P, 1], f32, kind="ExternalInput")
    woutr_d = nc.dram_tensor("woutr", [P, 16], f32, kind="ExternalInput")
    iota_d = nc.dram_tensor("iota", [P, P], bfl, kind="ExternalInput")
    ident_d = nc.dram_tensor("ident", [P, P], bfl, kind="ExternalInput")
    out_d = nc.dram_tensor("out", [nodep, 1], f32, kind="ExternalOutput")

    shard_t = [nc.dram_tensor(f"shard{l}", [nodep, P], bfl) for l in range(3)]
    table_t = [nc.dram_tensor(f"table{l}", [nodep * N_CORES, P], bfl,
                              addr_space="Shared") for l in range(3)]

    from contextlib import ExitStack as _ES
    with _ES() as _ctx:
        block = _ctx.enter_context(nc.Block())
        xt_sb = _ctx.enter_context(nc.sbuf_tensor("xt_sb", [P, KT, nodep], bfl))
        wall_sb = _ctx.enter_context(nc.sbuf_tensor("wall_sb", [P, KT, 112], bfl))
        w2_sb = _ctx.enter_context(nc.sbuf_tensor("w2_sb", [64, 32], bfl))
        w3_sb = _ctx.enter_context(nc.sbuf_tensor("w3_sb", [32, 16], bfl))
        ws13_sb = _ctx.enter_context(nc.sbuf_tensor("ws13_sb", [64, 16], bfl))
        biasin_sb = _ctx.enter_context(nc.sbuf_tensor("biasin_sb", [P, 176], f32))
        bias2_sb = _ctx.enter_context(nc.sbuf_tensor("bias2_sb", [P, 32], f32))
        bias3_sb = _ctx.enter_context(nc.sbuf_tensor("bias3_sb", [P, 16], f32))
        bout_sb = _ctx.enter_context(nc.sbuf_tensor("bout_sb", [P, 1], f32))
        woutr_sb = _ctx.enter_context(nc.sbuf_tensor("woutr_sb", [P, 16], f32))
        iota_sb = _ctx.enter_context(nc.sbuf_tensor("iota_sb", [P, P], bfl))
        ident_sb = _ctx.enter_context(nc.sbuf_tensor("ident_sb", [P, P], bfl))
        deg_sb = _ctx.enter_context(nc.sbuf_tensor("deg_sb", [P, nb], f32))
        dinv_sb = _ctx.enter_context(nc.sbuf_tensor("dinv_sb", [P, nb], f32))
        dstl_sb = _ctx.enter_context(nc.sbuf_tensor("dstl_sb", [P, n_chunks], f32))
        idx_sb = _ctx.enter_context(nc.sbuf_tensor("idx_sb", [P, NIB, ICOL], i16))
        gbuf = _ctx.enter_context(nc.sbuf_tensor("gbuf", [P, NGB, GCH, P], bfl))
        oh_sb = _ctx.enter_context(nc.sbuf_tensor("oh_sb", [P, NOH, P], bfl))
        skip_sb = _ctx.enter_context(nc.sbuf_tensor("skip_sb", [P, nb, 48], f32))
        skip13_sb = _ctx.enter_context(nc.sbuf_tensor("skip13_sb", [P, nb, 16], f32))
        xlt_sb = _ctx.enter_context(nc.sbuf_tensor("xlt_sb", [64, 2, P], bfl))
        stage_sb = _ctx.enter_context(nc.sbuf_tensor("stage_sb", [P, nb, 64], bfl))
        fin_sb = _ctx.enter_context(nc.sbuf_tensor("fin_sb", [P, 4, 64], f32))
        xl_sb = _ctx.enter_context(nc.sbuf_tensor("xl_sb", [P, 4, 64], bfl))
        x3w_sb = _ctx.enter_context(nc.sbuf_tensor("x3w_sb", [P, 16], f32))
        red_sb = _ctx.enter_context(nc.sbuf_tensor("red_sb", [P, 1], f32))
        dum_sb = _ctx.enter_context(nc.sbuf_tensor("dum_sb", [1, 1], f32))
        out_sb = _ctx.enter_context(nc.sbuf_tensor("out_sb", [P, nb], f32))
        ps_seg = _ctx.enter_context(nc.psum_tensor("ps_seg", [P, 5, 512], f32))
        ps_pj = _ctx.enter_context(nc.psum_tensor("ps_pj", [P, 2, 512], f32))
        ps_tp = _ctx.enter_context(nc.psum_tensor("ps_tp", [P, 1, 1024], bfl))
        io = _ctx.enter_context(nc.semaphore("io"))
        s_ib = [_ctx.enter_context(nc.semaphore(f"s_ib{j}"))
                for j in range(NIB)]
        s_q0 = _ctx.enter_context(nc.semaphore("s_q0"))
        s_q1 = _ctx.enter_context(nc.semaphore("s_q1"))
        s_q2 = _ctx.enter_context(nc.semaphore("s_q2"))
        s_q3 = _ctx.enter_context(nc.semaphore("s_q3"))
        s_oh = _ctx.enter_context(nc.semaphore("s_oh"))
        s_mm = _ctx.enter_context(nc.semaphore("s_mm"))
        s_f0 = _ctx.enter_context(nc.semaphore("s_f0"))
        s_f1 = _ctx.enter_context(nc.semaphore("s_f1"))
        s_f2 = _ctx.enter_context(nc.semaphore("s_f2"))
        s_f3 = _ctx.enter_context(nc.semaphore("s_f3"))
        s_f4 = _ctx.enter_context(nc.semaphore("s_f4"))
        s_relu = _ctx.enter_context(nc.semaphore("s_relu"))
        s_tpd = _ctx.enter_context(nc.semaphore("s_tpd"))
        s_pj = _ctx.enter_context(nc.semaphore("s_pj"))
        s_stg = _ctx.enter_context(nc.semaphore("s_stg"))
        s_shard = _ctx.enter_context(nc.semaphore("s_shard"))
        s_ag = _ctx.enter_context(nc.semaphore("s_ag"))
        s_dinv = _ctx.enter_context(nc.semaphore("s_dinv"))
        s_pem = _ctx.enter_context(nc.semaphore("s_pem"))
        s_dfin = _ctx.enter_context(nc.semaphore("s_dfin"))

        s_q = [s_q0, s_q1, s_q2, s_q3]
        s_f = [s_f0, s_f1, s_f2, s_f3, s_f4]

        # ------------------------------------------------------------ SYNC
        @block.sync
        def _(sync):
            loads = [
                (wall_sb[:, :, :],
                 wall_d[:, :].rearrange("(k p) n -> p k n", p=P)),
                (w2_sb[:, :], w2_d[:, :]),
                (w3_sb[:, :], w3_d[:, :]),
                (ws13_sb[:, :], ws13_d[:, :]),
                (biasin_sb[:, :], bias_d[:, :]),
                (bout_sb[:, :], bout_d[:, :]),
                (woutr_sb[:, :], woutr_d[:, :]),
                (iota_sb[:, :], iota_d[:, :]),
                (ident_sb[:, :], ident_d[:, :]),
                (deg_sb[:, :], deg_d[:, :]),
                (dstl_sb[:, :], dstl_d[:, :]),
                (xt_sb[:, :, :],
                 xt_d[:, :].rearrange("(k p) n -> p k n", p=P)),
            ]
            for ap, dram in loads:
                sync.dma_start(ap, dram).then_inc(io, 16)
            sync.wait_ge(s_stg, nb)
            sync.dma_start(
                shard_t[0].ap().rearrange("(b p) f -> p b f", p=P)[:, :, :64],
                stage_sb[:, :, :],
            ).then_inc(s_shard, 16)
            for l in range(3):
                for g in range(n_gath):
                    gi = l * n_gath + g
                    if gi >= NIB:
                        j = gi - NIB
                        sync.wait_ge(s_q[j % 4], 16 * (j // 4 + 1))
                    c0 = gpfx[g] * 8
                    ncol = (gpfx[g + 1] - gpfx[g]) * 8
                    sync.dma_start(
                        idx_sb[:, gi % NIB, :ncol],
                        idx_d[:, c0:c0 + ncol],
                    ).then_inc(s_ib[gi % NIB], 16)
                if l < 2:
                    sync.wait_ge(s_stg, (l + 2) * nb)
                    sync.dma_start(
                        shard_t[l + 1].ap().rearrange(
                            "(b p) f -> p b f", p=P)[:, :, :Fs[l + 1]],
                        stage_sb[:, :, :Fs[l + 1]],
                    ).then_inc(s_shard, 16)
            sync.wait_ge(s_stg, 4 * nb)
            with nc.allow_non_contiguous_dma(reason="tiny final output"):
                sync.dma_start(
                    out_d.ap().rearrange("(b p) one -> p (b one)", p=P),
                    out_sb[:, :],
                ).then_inc(io, 16)

        # ---------------------------------------------------------- GPSIMD
        @block.gpsimd
        def _(gp):
            gp.load_library(mlp)
            gp.wait_ge(s_shard, 16)
            gp.collective_compute(
                "AllGather", OP.bypass,
                replica_groups=[list(range(N_CORES))],
                ins=[shard_t[0][:, :]],
                outs=[table_t[0][:, :]],
            ).then_inc(s_ag, 1)
            for l in range(3):
                gp.wait_ge(s_ag, l + 1)
                for g in range(n_gath):
                    gi = l * n_gath + g
                    qq, tot = gathers[g]
                    gp.wait_ge(s_ib[gi % NIB], 16 * (gi // NIB + 1))
                    if gi >= NGB:
                        prev = gi - NGB
                        pl, pg = prev // n_gath, prev % n_gath
                        gp.wait_ge(s_mm, pl * n_chunks + gpfx[pg + 1])
                    q0 = qq * quad_rows
                    gp.dma_gather(
                        gbuf[:, gi % NGB, :tot // P, :],
                        table_t[l][q0:q0 + quad_rows, :],
                        idx_sb[:, gi % NIB, :tot // 16],
                        tot, tot, P,
                        single_packet=False,
                        queue_num=gi % 4,
                    ).then_inc(s_q[gi % 4], 16)
                if l < 2:
                    gp.wait_ge(s_shard, 16 * (l + 2))
                    gp.collective_compute(
                        "AllGather", OP.bypass,
                        replica_groups=[list(range(N_CORES))],
                        ins=[shard_t[l + 1][:, :]],
                        outs=[table_t[l + 1][:, :]],
                    ).then_inc(s_ag, 1)

        # -------------------------------------------------------------- PE
        @block.tensor
        def _(pe):
            pe.wait_ge(io, 16 * 12)
            for b in range(nb):
                u = PJ_ST1[b]
                if u >= 2:
                    pe.wait_ge(s_pj, u - 1)
                for kt in range(KT):
                    mmi = pe.matmul(
                        ps_pj[:, u % 2, :112],
                        xt_sb[:, kt, b * P:(b + 1) * P],
                        wall_sb[:, kt, :112],
                        start=(kt == 0), stop=(kt == KT - 1))
                mmi.then_inc(s_pem, 1)
            for l in range(3):
                F = Fs[l]
                fp = 0
                for ci, (b, st, fi) in enumerate(sched):
                    gc = l * n_chunks + ci
                    g = g_of_chunk[ci]
                    gi = l * n_gath + g
                    if ci == gpfx[g]:
                        pe.wait_ge(s_q[gi % 4], 16 * (gi // 4 + 1))
                    if ci in run_start:
                        pe.wait_ge(s_oh, l * n_chunks + run_end_of[ci] + 1)
                    if st and bank_use(l, b) > 0:
                        pe.wait_ge(s_f[bank_of(l, b)], bank_use(l, b))
                    pe.matmul(
                        ps_seg[:, bank_of(l, b), :F],
                        oh_sb[:, gc % NOH, :],
                        gbuf[:, gi % NGB, ci - gpfx[g], :F],
                        start=st, stop=fi,
                    ).then_inc(s_mm, 1)
                    if fi:
                        gl = l * nb + fp
                        if l < 2:
                            pe.wait_ge(s_relu, gl + 1)
                            if gl >= 1:
                                pe.wait_ge(s_tpd, gl)  # prev tp copied out
                            pe.transpose(
                                ps_tp[:F, 0, :P],
                                xl_sb[:, gl % 4, :F],
                                ident_sb[:, :],
                            ).then_inc(s_pem, 1)
                            pe.wait_ge(s_tpd, gl + 1)
                            xlt = xlt_sb[:F, gl % 2, :]
                            W_n = w2_sb[:, :] if l == 0 else w3_sb[:, :]
                            u = PJ1[gl]
                            if u >= 2:
                                pe.wait_ge(s_pj, u - 1)
                            pe.matmul(ps_pj[:, u % 2, :Fs[l + 1]], xlt, W_n,
                                      start=True, stop=True).then_inc(s_pem, 1)
                            if l == 0:
                                u = PJ2[gl]
                                if u >= 2:
                                    pe.wait_ge(s_pj, u - 1)
                                pe.matmul(ps_pj[:, u % 2, :16], xlt,
                                          ws13_sb[:, :], start=True,
                                          stop=True).then_inc(s_pem, 1)
                        fp += 1

        # ------------------------------------------------------------- DVE
        @block.vector
        def _(dve):
            dve.wait_ge(s_dinv, 1)
            dve.drain()
            dve.reciprocal(dinv_sb[:, :], dinv_sb[:, :])
            dve.drain().then_inc(s_dinv, 1)
            dve.wait_ge(io, 16 * 12)
            dve.tensor_tensor(bias2_sb[:, :], biasin_sb[:, 64:96],
                              biasin_sb[:, 96:128], OP.add)
            dve.tensor_tensor(bias3_sb[:, :], biasin_sb[:, 128:144],
                              biasin_sb[:, 144:160], OP.add)
            dve.drain()
            dve.tensor_tensor(bias3_sb[:, :], bias3_sb[:, :],
                              biasin_sb[:, 160:176], OP.add)
            dve.drain()
            for l in range(3):
                F = Fs[l]
                fp = 0
                for ci, (b, st, fi) in enumerate(sched):
                    gc = l * n_chunks + ci
                    if gc >= NOH:
                        dve.wait_ge(s_mm, gc - NOH + 1)
                    dve.tensor_scalar(oh_sb[:, gc % NOH, :], iota_sb[:, :],
                                      dstl_sb[:, ci:ci + 1], None,
                                      OP.is_equal).then_inc(s_oh, 1)
                    if fi:
                        gl = l * nb + fp
                        if gl >= 4:
                            dve.wait_ge(s_relu, gl - 3)   # fin buf free
                            dve.wait_ge(s_tpd, min(gl - 3, 2 * nb))  # xl free
                        dve.wait_ge(s_mm, gc + 1)
                        ft = fin_sb[:, gl % 4, :F]
                        dve.tensor_scalar(
                            ft, ps_seg[:, bank_of(l, b), :F],
                            dinv_sb[:, b:b + 1], None,
                            OP.mult).then_inc(s_f[bank_of(l, b)], 1)
                        dve.drain()
                        if l == 0:
                            dve.tensor_tensor(
                                ft, ft, biasin_sb[:, :64], OP.add)
                            dve.drain().then_inc(s_dfin, 1)
                        elif l == 1:
                            dve.tensor_tensor(ft, ft, bias2_sb[:, :], OP.add)
                            dve.drain()
                            dve.tensor_tensor(
                                ft, ft, skip_sb[:, b, :32], OP.add)
                            dve.drain().then_inc(s_dfin, 1)
                        else:
                            dve.tensor_tensor(ft, ft, bias3_sb[:, :], OP.add)
                            dve.drain()
                            dve.tensor_tensor(ft, ft, skip_sb[:, b, 32:48],
                                              OP.add)
                            dve.drain()
                            dve.tensor_tensor(
                                ft, ft, skip13_sb[:, b, :16], OP.add)
                            dve.drain().then_inc(s_dfin, 1)
                            # wout dot-product after ACT relu wrote x3w
                            dve.wait_ge(s_relu, gl + 1)
                            dve.tensor_tensor(x3w_sb[:, :], x3w_sb[:, :],
                                              woutr_sb[:, :], OP.mult)
                            dve.drain()
                            dve.tensor_reduce(
                                red_sb[:, :1], x3w_sb[:, :], axis=AX.X,
                                op=OP.add)
                            dve.drain().then_inc(s_dfin, 1)
                        fp += 1

        # ------------------------------------------------------------- ACT
        @block.scalar
        def _(act):
            act.memzero(dum_sb[:1, :1])
            act.wait_ge(io, 16 * 12)
            act.activation(dinv_sb[:, :], deg_sb[:, :],
                           AF.Sqrt).then_inc(s_dinv, 1)
            act.wait_ge(s_dinv, 2)
            for b in range(nb):
                act.wait_ge(s_pem, PEM_ST1[b])
                u = PJ_ST1[b]
                act.activation(stage_sb[:, b, :64],
                               ps_pj[:, u % 2, :64],
                               AF.Copy,
                               scale=dinv_sb[:, b:b + 1]).then_inc(s_stg, 1)
                act.activation(skip_sb[:, b, :48], ps_pj[:, u % 2, 64:112],
                               AF.Copy).then_inc(s_pj, 1)
            for l in range(3):
                F = Fs[l]
                fp = 0
                for ci, (b, st, fi) in enumerate(sched):
                    if not fi:
                        continue
                    gl = l * nb + fp
                    act.wait_ge(s_dfin, DF_A[gl])
                    if l < 2:
                        act.activation(xl_sb[:, gl % 4, :F],
                                       fin_sb[:, gl % 4, :F],
                                       AF.Relu).then_inc(s_relu, 1)
                        act.wait_ge(s_pem, PEM_TP[gl])
                        act.activation(xlt_sb[:F, gl % 2, :],
                                       ps_tp[:F, 0, :P],
                                       AF.Copy).then_inc(s_tpd, 1)
                        act.wait_ge(s_pem, PEM_P1[gl])
                        u = PJ1[gl]
                        act.activation(
                            stage_sb[:, b, :Fs[l + 1]],
                            ps_pj[:, u % 2, :Fs[l + 1]],
                            AF.Copy,
                            scale=dinv_sb[:, b:b + 1]).then_inc(s_stg, 1)
                        act.mul(dum_sb[:1, :1], dum_sb[:1, :1],
                                1.0).then_inc(s_pj, 1)
                        if l == 0:
                            act.wait_ge(s_pem, PEM_P2[gl])
                            u = PJ2[gl]
                            act.activation(
                                skip13_sb[:, b, :16],
                                ps_pj[:, u % 2, :16],
                                AF.Copy).then_inc(s_pj, 1)
                    else:
                        act.activation(x3w_sb[:, :], fin_sb[:, gl % 4, :16],
                                       AF.Relu).then_inc(s_relu, 1)
                        act.wait_ge(s_dfin, DF_B[gl])
                        act.activation(
                            out_sb[:, b:b + 1], red_sb[:, :1],
                            AF.Sigmoid,
                            bias=bout_sb[:, :1]).then_inc(s_stg, 1)
                    fp += 1

    nc.compile()
    return nc


def kernel(**inputs):
    x = np.asarray(inputs["x"], np.float32)
    edge_index = np.asarray(inputs["edge_index"])
    n_nodes, F_IN = x.shape
    cores, common = _prep(edge_index, n_nodes)
    shard, nodep = common["shard"], common["nodep"]

    nc = build_program(common, F_IN)

    W1 = np.asarray(inputs["W1"], np.float32)
    Ws02 = np.asarray(inputs["Ws02"], np.float32)
    Ws03 = np.asarray(inputs["Ws03"], np.float32)
    wall = np.concatenate([W1, Ws02, Ws03], axis=1).astype(bf16)  # [F_IN,112]
    bias = np.concatenate([
        np.asarray(inputs["b1"], np.float32),
        np.asarray(inputs["b2"], np.float32),
        np.asarray(inputs["bs02"], np.float32),
        np.asarray(inputs["b3"], np.float32),
        np.asarray(inputs["bs03"], np.float32),
        np.asarray(inputs["bs13"], np.float32),
    ])
    bias_rep = np.ascontiguousarray(np.tile(bias[None, :], (P, 1)))
    bout_rep = np.ascontiguousarray(
        np.tile(np.asarray(inputs["bout"], np.float32)[None, :], (P, 1)))
    woutr = np.ascontiguousarray(
        np.tile(np.asarray(inputs["Wout"], np.float32).reshape(1, 16), (P, 1)))
    iota = np.ascontiguousarray(
        np.tile(np.arange(P, dtype=np.float32)[None, :], (P, 1)).astype(bf16))
    ident = np.ascontiguousarray(np.eye(P, dtype=np.float32).astype(bf16))
    w2 = np.asarray(inputs["W2"], np.float32).astype(bf16)
    w3 = np.asarray(inputs["W3"], np.float32).astype(bf16)
    ws13 = np.asarray(inputs["Ws13"], np.float32).astype(bf16)

    in_maps = []
    for c in range(N_CORES):
        xs = np.zeros((nodep, F_IN), np.float32)
        xs[:shard] = x[c * shard:(c + 1) * shard]
        xt = np.ascontiguousarray(xs.T.astype(bf16))  # [F_IN, nodep]
        in_maps.append(dict(
            xt=xt, idx=cores[c]["idx"], dstl=cores[c]["dstl"],
            deg=cores[c]["deg"], wall=wall, w2=w2, w3=w3, ws13=ws13,
            bias=bias_rep, bout=bout_rep, woutr=woutr, iota=iota,
            ident=ident,
        ))
    res = run_bass_kernel_spmd(nc, in_maps, list(range(N_CORES)))
    out = np.concatenate(
        [res.results[c]["out"][:shard] for c in range(N_CORES)], axis=0)
    return out.astype(np.float32)



# revision 3
# speedup vs baseline: 1592.0360x; 1592.0360x over previous
"""Trainium2 Bass kernel for a 3-layer dense GCN (nn_DenseGCN3Layer).

Node/dst sharding over 8 NeuronCores. Per layer: each core projects its
local shard to table_l = dinv*(x_{l-1} @ W_l) in bf16, AllGathers the full
table into local HBM, then fetches per-edge 256B source rows with 4-queue
SWDGE dma_gather; TensorE one-hot matmuls segment-sum edges into PSUM per
128-dst-node block; DVE/ACT finish ops apply dinv, bias, skips, ReLU.
Host work is integer edge indexing/sorting, dtype casts, layout packing.
"""
import numpy as np
import ml_dtypes

import concourse.bacc as bacc
import concourse.bass as bass
import concourse.mybir as mybir
from concourse._compat import cdiv
from concourse.bass_utils import run_bass_kernel_spmd
from concourse.library_config import mlp

bf16 = ml_dtypes.bfloat16
f32 = mybir.dt.float32
bfl = mybir.dt.bfloat16
i16 = mybir.dt.int16
AF = mybir.ActivationFunctionType
OP = mybir.AluOpType
AX = mybir.AxisListType

N_CORES = 8
P = 128
NOH = 40    # one-hot buffers
NGB = 4     # gather buffers
NIB = 5     # idx slice buffers
MAXG = 4608  # max slots per gather instruction
GCH = MAXG // P
Fs = [64, 32, 16]


def _prep(edge_index, n_nodes):
    src = np.asarray(edge_index[0], np.int64)
    dst = np.asarray(edge_index[1], np.int64)
    loops = np.arange(n_nodes, dtype=np.int64)
    src = np.concatenate([src, loops])
    dst = np.concatenate([dst, loops])
    deg = np.bincount(dst, minlength=n_nodes).astype(np.float64)

    shard = n_nodes // N_CORES
    nodep = cdiv(shard, P) * P
    if nodep == shard:
        nodep += P
    n_blocks = nodep // P
    quad_rows = 2 * nodep
    assert quad_rows < 32768
    nq = 4

    core_of = dst // shard
    per_core = []
    run_len = np.zeros((n_blocks, nq), np.int64)
    for c in range(N_CORES):
        m = core_of == c
        s = src[m]
        dloc = dst[m] - c * shard
        blk = dloc // P
        q = s // (2 * shard)
        idxq = ((s // shard) % 2) * nodep + (s % shard)
        order = np.lexsort((idxq, q, blk))
        blk, q, idxq, dloc = blk[order], q[order], idxq[order], dloc[order]
        runs = {}
        for b in range(n_blocks):
            mb = blk == b
            qb, ib, db = q[mb], idxq[mb], dloc[mb] % P
            for qq in range(nq):
                m2 = qb == qq
                runs[(b, qq)] = (ib[m2], db[m2])
                run_len[b, qq] = max(run_len[b, qq], int(m2.sum()))
        deg_c = np.full(nodep, 1e30)
        deg_c[:shard] = deg[c * shard:(c + 1) * shard]
        per_core.append((runs, deg_c))

    run_pad = ((run_len + P) // P) * P

    sbs, cur, cur_tot = [], [], 0
    for b in range(n_blocks):
        mx = int(run_pad[b].max())
        if cur and cur_tot + mx > MAXG:
            sbs.append(cur)
            cur, cur_tot = [], 0
        cur.append(b)
        cur_tot += mx
    if cur:
        sbs.append(cur)

    gathers, sched = [], []
    for sb in sbs:
        for qq in range(nq):
            tot = 0
            for b in sb:
                npad = int(run_pad[b, qq])
                tot += npad
                for ci in range(npad // P):
                    sched.append((b, qq == 0 and ci == 0,
                                  qq == nq - 1 and ci == npad // P - 1))
            gathers.append((qq, tot))

    cores = []
    for c in range(N_CORES):
        runs, deg_c = per_core[c]
        idx_s, dstl_s = [], []
        for sb in sbs:
            for qq in range(nq):
                for b in sb:
                    ib, db = runs[(b, qq)]
                    pad = int(run_pad[b, qq]) - len(ib)
                    idx_s.append(np.concatenate(
                        [ib, np.full(pad, nodep - 1, np.int64)]))
                    dstl_s.append(np.concatenate(
                        [db, np.full(pad, 254, np.int64)]))
        idx_s = np.concatenate(idx_s)
        dstl_s = np.concatenate(dstl_s)
        w = idx_s.reshape(-1, 16).T.astype(np.int16)
        cores.append(dict(
            idx=np.ascontiguousarray(np.tile(w, (8, 1))),
            dstl=np.ascontiguousarray(
                dstl_s.reshape(-1, P).T.astype(np.float32)),
            deg=np.ascontiguousarray(
                deg_c.reshape(n_blocks, P).T.astype(np.float32)),
        ))
    common = dict(gathers=gathers, sched=sched, n_slots=len(idx_s),
                  shard=shard, nodep=nodep, n_blocks=n_blocks, nq=nq,
                  quad_rows=quad_rows)
    return cores, common


def build_program(common, F_IN):
    nodep = common["nodep"]
    n_blocks, nq = common["n_blocks"], common["nq"]
    gathers, sched = common["gathers"], common["sched"]
    n_slots = common["n_slots"]
    n_chunks = n_slots // P
    quad_rows = common["quad_rows"]
    KT = F_IN // P
    assert F_IN % P == 0
    n_gath = len(gathers)
    nb = n_blocks
    ICOL = MAXG // 16

    gpfx = [0]
    for (_, tot) in gathers:
        gpfx.append(gpfx[-1] + tot // P)
    assert gpfx[-1] == n_chunks
    g_of_chunk = []
    for g, (_, tot) in enumerate(gathers):
        g_of_chunk += [g] * (tot // P)
    # maximal consecutive same-(gather, block) runs; PE waits s_oh per run
    run_start = []
    run_end_of = [0] * n_chunks
    i = 0
    while i < n_chunks:
        j = i
        while (j + 1 < n_chunks and g_of_chunk[j + 1] == g_of_chunk[i]
               and sched[j + 1][0] == sched[i][0]):
            j += 1
        for k in range(i, j + 1):
            run_end_of[k] = j
        run_start.append(i)
        i = j + 1
    run_start = set(run_start)
    assert max(run_end_of[i] - i for i in run_start) < NOH - 2

    # ---------------- pre-pass: event tables for sem targets -------------
    # pem: PE misc ops (stage1 block-mms count 1, transposes, projections)
    # pj:  ps_pj bank uses (global, ping-pong over 2 banks)
    # tpd: transpose copy-out count; relu: finish relu count
    # dfin: DVE finish-chain inc count; stg: staging-write count
    PEM_ST1 = {}
    PJ_ST1 = {}
    PEM_TP = {}
    PEM_P1 = {}
    PEM_P2 = {}
    PJ1 = {}
    PJ2 = {}
    DF_A = {}
    DF_B = {}
    pem = pj = dfin = 0
    for b in range(nb):
        pem += 1
        PEM_ST1[b] = pem
        PJ_ST1[b] = pj
        pj += 1
    fins = [[] for _ in range(3)]
    for l in range(3):
        fp = 0
        for ci, (b, st, fi) in enumerate(sched):
            if fi:
                fins[l].append((b, ci))
                gl = l * nb + fp
                dfin += 1
                DF_A[gl] = dfin
                if l < 2:
                    pem += 1
                    PEM_TP[gl] = pem
                    PJ1[gl] = pj
                    pj += 1
                    pem += 1
                    PEM_P1[gl] = pem
                    if l == 0:
                        PJ2[gl] = pj
                        pj += 1
                        pem += 1
                        PEM_P2[gl] = pem
                else:
                    dfin += 1
                    DF_B[gl] = dfin
                fp += 1

    def bank_of(l, b):
        return (l * nb + b) % 5

    def bank_use(l, b):
        return (l * nb + b) // 5

    nc = bacc.Bacc("TRN2", target_bir_lowering=False, debug=False,
                   num_devices=N_CORES, num_swdge_queues=4)

    xt_d = nc.dram_tensor("xt", [F_IN, nodep], bfl, kind="ExternalInput")
    idx_d = nc.dram_tensor("idx", [P, n_slots // 16], i16, kind="ExternalInput")
    dstl_d = nc.dram_tensor("dstl", [P, n_chunks], f32, kind="ExternalInput")
    deg_d = nc.dram_tensor("deg", [P, nb], f32, kind="ExternalInput")
    wall_d = nc.dram_tensor("wall", [F_IN, 112], bfl, kind="ExternalInput")
    w2_d = nc.dram_tensor("w2", [64, 32], bfl, kind="ExternalInput")
    w3_d = nc.dram_tensor("w3", [32, 16], bfl, kind="ExternalInput")
    ws13_d = nc.dram_tensor("ws13", [64, 16], bfl, kind="ExternalInput")
    bias_d = nc.dram_tensor("bias", [P, 176], f32, kind="ExternalInput")
    bout_d = nc.dram_tensor("bout", [P, 1], f32, kind="ExternalInput")
    woutr_d = nc.dram_tensor("woutr", [P, 16], f32, kind="ExternalInput")
    iota_d = nc.dram_tensor("iota", [P, P], bfl, kind="ExternalInput")
    ident_d = nc.dram_tensor("ident", [P, P], bfl, kind="ExternalInput")
    out_d = nc.dram_tensor("out", [nodep, 1], f32, kind="ExternalOutput")

    shard_t = [nc.dram_tensor(f"shard{l}", [nodep, P], bfl) for l in range(3)]
    table_t = [nc.dram_tensor(f"table{l}", [nodep * N_CORES, P], bfl,
                              addr_space="Shared") for l in range(3)]

    from contextlib import ExitStack as _ES
    with _ES() as _ctx:
        block = _ctx.enter_context(nc.Block())
        xt_sb = _ctx.enter_context(nc.sbuf_tensor("xt_sb", [P, KT, nodep], bfl))
        wall_sb = _ctx.enter_context(nc.sbuf_tensor("wall_sb", [P, KT, 112], bfl))
        w2_sb = _ctx.enter_context(nc.sbuf_tensor("w2_sb", [64, 32], bfl))
        w3_sb = _ctx.enter_context(nc.sbuf_tensor("w3_sb", [32, 16], bfl))
        ws13_sb = _ctx.enter_context(nc.sbuf_tensor("ws13_sb", [64, 16], bfl))
        biasin_sb = _ctx.enter_context(nc.sbuf_tensor("biasin_sb", [P, 176], f32))
        bias2_sb = _ctx.enter_context(nc.sbuf_tensor("bias2_sb", [P, 32], f32))
        bias3_sb = _ctx.enter_context(nc.sbuf_tensor("bias3_sb", [P, 16], f32))
        bout_sb = _ctx.enter_context(nc.sbuf_tensor("bout_sb", [P, 1], f32))
        woutr_sb = _ctx.enter_context(nc.sbuf_tensor("woutr_sb", [P, 16], f32))
        iota_sb = _ctx.enter_context(nc.sbuf_tensor("iota_sb", [P, P], bfl))
        ident_sb = _ctx.enter_context(nc.sbuf_tensor("ident_sb", [P, P], bfl))
        deg_sb = _ctx.enter_context(nc.sbuf_tensor("deg_sb", [P, nb], f32))
        dinv_sb = _ctx.enter_context(nc.sbuf_tensor("dinv_sb", [P, nb], f32))
        dstl_sb = _ctx.enter_context(nc.sbuf_tensor("dstl_sb", [P, n_chunks], f32))
        idx_sb = _ctx.enter_context(nc.sbuf_tensor("idx_sb", [P, NIB, ICOL], i16))
        gbuf = _ctx.enter_context(nc.sbuf_tensor("gbuf", [P, NGB, GCH, P], bfl))
        oh_sb = _ctx.enter_context(nc.sbuf_tensor("oh_sb", [P, NOH, P], bfl))
        skip_sb = _ctx.enter_context(nc.sbuf_tensor("skip_sb", [P, nb, 48], f32))
        skip13_sb = _ctx.enter_context(nc.sbuf_tensor("skip13_sb", [P, nb, 16], f32))
        xlt_sb = _ctx.enter_context(nc.sbuf_tensor("xlt_sb", [64, 2, P], bfl))
        stage_sb = _ctx.enter_context(nc.sbuf_tensor("stage_sb", [P, nb, 64], bfl))
        fin_sb = _ctx.enter_context(nc.sbuf_tensor("fin_sb", [P, 4, 64], f32))
        xl_sb = _ctx.enter_context(nc.sbuf_tensor("xl_sb", [P, 4, 64], bfl))
        x3w_sb = _ctx.enter_context(nc.sbuf_tensor("x3w_sb", [P, 16], f32))
        red_sb = _ctx.enter_context(nc.sbuf_tensor("red_sb", [P, 1], f32))
        dum_sb = _ctx.enter_context(nc.sbuf_tensor("dum_sb", [1, 1], f32))
        out_sb = _ctx.enter_context(nc.sbuf_tensor("out_sb", [P, nb], f32))
        ps_seg = _ctx.enter_context(nc.psum_tensor("ps_seg", [P, 5, 512], f32))
        ps_pj = _ctx.enter_context(nc.psum_tensor("ps_pj", [P, 2, 512], f32))
        ps_tp = _ctx.enter_context(nc.psum_tensor("ps_tp", [P, 1, 1024], bfl))
        io = _ctx.enter_context(nc.semaphore("io"))
        s_ib = [_ctx.enter_context(nc.semaphore(f"s_ib{j}"))
                for j in range(NIB)]
        s_q0 = _ctx.enter_context(nc.semaphore("s_q0"))
        s_q1 = _ctx.enter_context(nc.semaphore("s_q1"))
        s_q2 = _ctx.enter_context(nc.semaphore("s_q2"))
        s_q3 = _ctx.enter_context(nc.semaphore("s_q3"))
        s_oh = _ctx.enter_context(nc.semaphore("s_oh"))
        s_mm = _ctx.enter_context(nc.semaphore("s_mm"))
        s_f0 = _ctx.enter_context(nc.semaphore("s_f0"))
        s_f1 = _ctx.enter_context(nc.semaphore("s_f1"))
        s_f2 = _ctx.enter_context(nc.semaphore("s_f2"))
        s_f3 = _ctx.enter_context(nc.semaphore("s_f3"))
        s_f4 = _ctx.enter_context(nc.semaphore("s_f4"))
        s_relu = _ctx.enter_context(nc.semaphore("s_relu"))
        s_tpd = _ctx.enter_context(nc.semaphore("s_tpd"))
        s_pj = _ctx.enter_context(nc.semaphore("s_pj"))
        s_stg = _ctx.enter_context(nc.semaphore("s_stg"))
        s_shard = _ctx.enter_context(nc.semaphore("s_shard"))
        s_ag = _ctx.enter_context(nc.semaphore("s_ag"))
        s_dinv = _ctx.enter_context(nc.semaphore("s_dinv"))
        s_pem = _ctx.enter_context(nc.semaphore("s_pem"))
        s_dfin = _ctx.enter_context(nc.semaphore("s_dfin"))

        s_q = [s_q0, s_q1, s_q2, s_q3]
        s_f = [s_f0, s_f1, s_f2, s_f3, s_f4]

        # ------------------------------------------------------------ SYNC
        @block.sync
        def _(sync):
            loads = [
                (wall_sb[:, :, :],
                 wall_d[:, :].rearrange("(k p) n -> p k n", p=P)),
                (w2_sb[:, :], w2_d[:, :]),
                (w3_sb[:, :], w3_d[:, :]),
                (ws13_sb[:, :], ws13_d[:, :]),
                (biasin_sb[:, :], bias_d[:, :]),
                (bout_sb[:, :], bout_d[:, :]),
                (woutr_sb[:, :], woutr_d[:, :]),
                (iota_sb[:, :], iota_d[:, :]),
                (ident_sb[:, :], ident_d[:, :]),
                (deg_sb[:, :], deg_d[:, :]),
                (dstl_sb[:, :], dstl_d[:, :]),
                (xt_sb[:, :, :],
                 xt_d[:, :].rearrange("(k p) n -> p k n", p=P)),
            ]
            for ap, dram in loads:
                sync.dma_start(ap, dram).then_inc(io, 16)
            sync.wait_ge(s_stg, nb)
            sync.dma_start(
                shard_t[0].ap().rearrange("(b p) f -> p b f", p=P)[:, :, :64],
                stage_sb[:, :, :],
            ).then_inc(s_shard, 16)
            for l in range(3):
                for g in range(n_gath):
                    gi = l * n_gath + g
                    if gi >= NIB:
                        j = gi - NIB
                        sync.wait_ge(s_q[j % 4], 16 * (j // 4 + 1))
                    c0 = gpfx[g] * 8
                    ncol = (gpfx[g + 1] - gpfx[g]) * 8
                    sync.dma_start(
                        idx_sb[:, gi % NIB, :ncol],
                        idx_d[:, c0:c0 + ncol],
                    ).then_inc(s_ib[gi % NIB], 16)
                if l < 2:
                    sync.wait_ge(s_stg, (l + 2) * nb)
                    sync.dma_start(
                        shard_t[l + 1].ap().rearrange(
                            "(b p) f -> p b f", p=P)[:, :, :Fs[l + 1]],
                        stage_sb[:, :, :Fs[l + 1]],
                    ).then_inc(s_shard, 16)
            sync.wait_ge(s_stg, 4 * nb)
            with nc.allow_non_contiguous_dma(reason="tiny final output"):
                sync.dma_start(
                    out_d.ap().rearrange("(b p) one -> p (b one)", p=P),
                    out_sb[:, :],
                ).then_inc(io, 16)

        # ---------------------------------------------------------- GPSIMD
        @block.gpsimd
        def _(gp):
            gp.load_library(mlp)
            gp.wait_ge(s_shard, 16)
            gp.collective_compute(
                "AllGather", OP.bypass,
                replica_groups=[list(range(N_CORES))],
                ins=[shard_t[0][:, :]],
                outs=[table_t[0][:, :]],
            ).then_inc(s_ag, 1)
            for l in range(3):
                gp.wait_ge(s_ag, l + 1)
                for g in range(n_gath):
                    gi = l * n_gath + g
                    qq, tot = gathers[g]
                    gp.wait_ge(s_ib[gi % NIB], 16 * (gi // NIB + 1))
                    if gi >= NGB:
                        prev = gi - NGB
                        pl, pg = prev // n_gath, prev % n_gath
                        gp.wait_ge(s_mm, pl * n_chunks + gpfx[pg + 1])
                    q0 = qq * quad_rows
                    gp.dma_gather(
                        gbuf[:, gi % NGB, :tot // P, :],
                        table_t[l][q0:q0 + quad_rows, :],
                        idx_sb[:, gi % NIB, :tot // 16],
                        tot, tot, P,
                        single_packet=False,
                        queue_num=gi % 4,
                    ).then_inc(s_q[gi % 4], 16)
                if l < 2:
                    gp.wait_ge(s_shard, 16 * (l + 2))
                    gp.collective_compute(
                        "AllGather", OP.bypass,
                        replica_groups=[list(range(N_CORES))],
                        ins=[shard_t[l + 1][:, :]],
                        outs=[table_t[l + 1][:, :]],
                    ).then_inc(s_ag, 1)

        # -------------------------------------------------------------- PE
        @block.tensor
        def _(pe):
            pe.wait_ge(io, 16 * 12)
            for b in range(nb):
                u = PJ_ST1[b]
                if u >= 2:
                    pe.wait_ge(s_pj, u - 1)
                for kt in range(KT):
                    mmi = pe.matmul(
                        ps_pj[:, u % 2, :112],
                        xt_sb[:, kt, b * P:(b + 1) * P],
                        wall_sb[:, kt, :112],
                        start=(kt == 0), stop=(kt == KT - 1))
                mmi.then_inc(s_pem, 1)
            for l in range(3):
                F = Fs[l]
                fp = 0
                for ci, (b, st, fi) in enumerate(sched):
                    gc = l * n_chunks + ci
                    g = g_of_chunk[ci]
                    gi = l * n_gath + g
                    if ci == gpfx[g]:
                        pe.wait_ge(s_q[gi % 4], 16 * (gi // 4 + 1))
                    if ci in run_start:
                        pe.wait_ge(s_oh, l * n_chunks + run_end_of[ci] + 1)
                    if st and bank_use(l, b) > 0:
                        pe.wait_ge(s_f[bank_of(l, b)], bank_use(l, b))
                    pe.matmul(
                        ps_seg[:, bank_of(l, b), :F],
                        oh_sb[:, gc % NOH, :],
                        gbuf[:, gi % NGB, ci - gpfx[g], :F],
                        start=st, stop=fi,
                    ).then_inc(s_mm, 1)
                    if fi:
                        gl = l * nb + fp
                        if l < 2:
                            pe.wait_ge(s_relu, gl + 1)
                            if gl >= 1:
                                pe.wait_ge(s_tpd, gl)  # prev tp copied out
                            pe.transpose(
                                ps_tp[:F, 0, :P],
                                xl_sb[:, gl % 4, :F],
                                ident_sb[:, :],
                            ).then_inc(s_pem, 1)
                            pe.wait_ge(s_tpd, gl + 1)
                            xlt = xlt_sb[:F, gl % 2, :]
                            W_n = w2_sb[:, :] if l == 0 else w3_sb[:, :]
                            u = PJ1[gl]
                            if u >= 2:
                                pe.wait_ge(s_pj, u - 1)
                            pe.matmul(ps_pj[:, u % 2, :Fs[l + 1]], xlt, W_n,
                                      start=True, stop=True).then_inc(s_pem, 1)
                            if l == 0:
                                u = PJ2[gl]
                                if u >= 2:
                                    pe.wait_ge(s_pj, u - 1)
                                pe.matmul(ps_pj[:, u % 2, :16], xlt,
                                          ws13_sb[:, :], start=True,
                                          stop=True).then_inc(s_pem, 1)
                        fp += 1

        # ------------------------------------------------------------- DVE
        @block.vector
        def _(dve):
            dve.wait_ge(s_dinv, 1)
            dve.drain()
            dve.reciprocal(dinv_sb[:, :], dinv_sb[:, :])
            dve.drain().then_inc(s_dinv, 1)
            dve.wait_ge(io, 16 * 12)
            dve.tensor_tensor(bias2_sb[:, :], biasin_sb[:, 64:96],
                              biasin_sb[:, 96:128], OP.add)
            dve.tensor_tensor(bias3_sb[:, :], biasin_sb[:, 128:144],
                              biasin_sb[:, 144:160], OP.add)
            dve.drain()
            dve.tensor_tensor(bias3_sb[:, :], bias3_sb[:, :],
                              biasin_sb[:, 160:176], OP.add)
            dve.drain()
            for l in range(3):
                F = Fs[l]
                fp = 0
                for ci, (b, st, fi) in enumerate(sched):
                    gc = l * n_chunks + ci
                    if gc >= NOH:
                        dve.wait_ge(s_mm, gc - NOH + 1)
                    dve.tensor_scalar(oh_sb[:, gc % NOH, :], iota_sb[:, :],
                                      dstl_sb[:, ci:ci + 1], None,
                                      OP.is_equal).then_inc(s_oh, 1)
                    if fi:
                        gl = l * nb + fp
                        if gl >= 4:
                            dve.wait_ge(s_relu, gl - 3)   # fin buf free
                            dve.wait_ge(s_tpd, min(gl - 3, 2 * nb))  # xl free
                        dve.wait_ge(s_mm, gc + 1)
                        ft = fin_sb[:, gl % 4, :F]
                        dve.tensor_scalar(
                            ft, ps_seg[:, bank_of(l, b), :F],
                            dinv_sb[:, b:b + 1], None,
                            OP.mult).then_inc(s_f[bank_of(l, b)], 1)
                        dve.drain()
                        if l == 0:
                            dve.tensor_tensor(
                                ft, ft, biasin_sb[:, :64], OP.add)
                            dve.drain().then_inc(s_dfin, 1)
                        elif l == 1:
                            dve.tensor_tensor(ft, ft, bias2_sb[:, :], OP.add)
                            dve.drain()
                            dve.tensor_tensor(
                                ft, ft, skip_sb[:, b, :32], OP.add)
                            dve.drain().then_inc(s_dfin, 1)
                        else:
                            dve.tensor_tensor(ft, ft, bias3_sb[:, :], OP.add)
                            dve.drain()
                            dve.tensor_tensor(ft, ft, skip_sb[:, b, 32:48],
                                              OP.add)
                            dve.drain()
                            dve.tensor_tensor(
                                ft, ft, skip13_sb[:, b, :16], OP.add)
                            dve.drain().then_inc(s_dfin, 1)
                            # wout dot-product after ACT relu wrote x3w
                            dve.wait_ge(s_relu, gl + 1)
                            dve.tensor_tensor(x3w_sb[:, :], x3w_sb[:, :],
                                              woutr_sb[:, :], OP.mult)
                            dve.drain()
                            dve.tensor_reduce(
                                red_sb[:, :1], x3w_sb[:, :], axis=AX.X,
                                op=OP.add)
                            dve.drain().then_inc(s_dfin, 1)
                        fp += 1

        # ------------------------------------------------------------- ACT
        @block.scalar
        def _(act):
            act.memzero(dum_sb[:1, :1])
            act.wait_ge(io, 16 * 12)
            act.activation(dinv_sb[:, :], deg_sb[:, :],
                           AF.Sqrt).then_inc(s_dinv, 1)
            act.wait_ge(s_dinv, 2)
            for b in range(nb):
                act.wait_ge(s_pem, PEM_ST1[b])
                u = PJ_ST1[b]
                act.activation(stage_sb[:, b, :64],
                               ps_pj[:, u % 2, :64],
                               AF.Copy,
                               scale=dinv_sb[:, b:b + 1]).then_inc(s_stg, 1)
                act.activation(skip_sb[:, b, :48], ps_pj[:, u % 2, 64:112],
                               AF.Copy).then_inc(s_pj, 1)
            for l in range(3):
                F = Fs[l]
                fp = 0
                for ci, (b, st, fi) in enumerate(sched):
                    if not fi:
                        continue
                    gl = l * nb + fp
                    act.wait_ge(s_dfin, DF_A[gl])
                    if l < 2:
                        act.activation(xl_sb[:, gl % 4, :F],
                                       fin_sb[:, gl % 4, :F],
                                       AF.Relu).then_inc(s_relu, 1)
                        act.wait_ge(s_pem, PEM_TP[gl])
                        act.activation(xlt_sb[:F, gl % 2, :],
                                       ps_tp[:F, 0, :P],
                                       AF.Copy).then_inc(s_tpd, 1)
                        act.wait_ge(s_pem, PEM_P1[gl])
                        u = PJ1[gl]
                        act.activation(
                            stage_sb[:, b, :Fs[l + 1]],
                            ps_pj[:, u % 2, :Fs[l + 1]],
                            AF.Copy,
                            scale=dinv_sb[:, b:b + 1]).then_inc(s_stg, 1)
                        act.mul(dum_sb[:1, :1], dum_sb[:1, :1],
                                1.0).then_inc(s_pj, 1)
                        if l == 0:
                            act.wait_ge(s_pem, PEM_P2[gl])
                            u = PJ2[gl]
                            act.activation(
                                skip13_sb[:, b, :16],
                                ps_pj[:, u % 2, :16],
                                AF.Copy).then_inc(s_pj, 1)
                    else:
                        act.activation(x3w_sb[:, :], fin_sb[:, gl % 4, :16],
                                       AF.Relu).then_inc(s_relu, 1)
                        act.wait_ge(s_dfin, DF_B[gl])
                        act.activation(
                            out_sb[:, b:b + 1], red_sb[:, :1],
                            AF.Sigmoid,
                            bias=bout_sb[:, :1]).then_inc(s_stg, 1)
                    fp += 1

    nc.compile()
    return nc


def prepare(inputs):
    x = np.asarray(inputs["x"], np.float32)
    edge_index = np.asarray(inputs["edge_index"])
    n_nodes, F_IN = x.shape
    cores, common = _prep(edge_index, n_nodes)
    shard, nodep = common["shard"], common["nodep"]

    nc = build_program(common, F_IN)

    W1 = np.asarray(inputs["W1"], np.float32)
    Ws02 = np.asarray(inputs["Ws02"], np.float32)
    Ws03 = np.asarray(inputs["Ws03"], np.float32)
    wall = np.concatenate([W1, Ws02, Ws03], axis=1).astype(bf16)  # [F_IN,112]
    bias = np.concatenate([
        np.asarray(inputs["b1"], np.float32),
        np.asarray(inputs["b2"], np.float32),
        np.asarray(inputs["bs02"], np.float32),
        np.asarray(inputs["b3"], np.float32),
        np.asarray(inputs["bs03"], np.float32),
        np.asarray(inputs["bs13"], np.float32),
    ])
    bias_rep = np.ascontiguousarray(np.tile(bias[None, :], (P, 1)))
    bout_rep = np.ascontiguousarray(
        np.tile(np.asarray(inputs["bout"], np.float32)[None, :], (P, 1)))
    woutr = np.ascontiguousarray(
        np.tile(np.asarray(inputs["Wout"], np.float32).reshape(1, 16), (P, 1)))
    iota = np.ascontiguousarray(
        np.tile(np.arange(P, dtype=np.float32)[None, :], (P, 1)).astype(bf16))
    ident = np.ascontiguousarray(np.eye(P, dtype=np.float32).astype(bf16))
    w2 = np.asarray(inputs["W2"], np.float32).astype(bf16)
    w3 = np.asarray(inputs["W3"], np.float32).astype(bf16)
    ws13 = np.asarray(inputs["Ws13"], np.float32).astype(bf16)

    in_maps = []
    for c in range(N_CORES):
        xs = np.zeros((nodep, F_IN), np.float32)
        xs[:shard] = x[c * shard:(c + 1) * shard]
        xt = np.ascontiguousarray(xs.T.astype(bf16))  # [F_IN, nodep]
        in_maps.append(dict(
            xt=xt, idx=cores[c]["idx"], dstl=cores[c]["dstl"],
            deg=cores[c]["deg"], wall=wall, w2=w2, w3=w3, ws13=ws13,
            bias=bias_rep, bout=bout_rep, woutr=woutr, iota=iota,
            ident=ident,
        ))
    return dict(nc=nc, in_maps=in_maps, common=common)


def finish(prep, results):
    shard = prep["common"]["shard"]
    out = np.concatenate(
        [results[c]["out"][:shard] for c in range(N_CORES)], axis=0)
    return out.astype(np.float32)


def kernel(**inputs):
    prep = prepare(inputs)
    res = run_bass_kernel_spmd(
        prep["nc"], prep["in_maps"], list(range(N_CORES)))
    return finish(prep, res.results)

